# revision 1
# baseline (speedup 1.0000x reference)
"""nn_CharRNN Trainium2 Bass kernel.

LSTM (B=64, T=512, I=128, H=1024) + final fc, data-parallel across 8
NeuronCores (8 sequences per core, weights replicated on device via a
one-shot AllGather of host-sharded tensors).

The execution environment dispatches instructions at a large fixed cost,
so the kernel minimizes instruction count:
  - gates in out[batch, gates] orientation: moving operand N=512 covers
    8 weight tiles per matmul -> 64 matmuls per LSTM step (8 K-chunks x
    8 PSUM banks), all bf16 (fp32 PSUM accumulate).
  - wide elementwise: one Sigmoid over [8, 3072] (gate order i,f,o,g),
    one Tanh for g, three VectorE ops for the cell update.
  - h is returned to [hdim, batch] layout with 8 SBUF->SBUF DMA
    transposes writing a [128, 16*8] tile, then one strided copy into the
    bf16 h-history buffer (which doubles as next step's stationary and
    the fc rhs).
  - x-projection + bias precomputed on device into DRAM (256 matmuls),
    streamed back one step per DMA and added to PSUM by VectorE.
  - fc runs once at the end over the h history (k-outer, 8 live PSUM
    banks, N=512).
  - redundant LDWEIGHTS are removed post-compile: consecutive matmuls
    that reuse the same stationary keep a single weight load.

Measured end-to-end error vs the fp32 reference: ~4e-3 frobenius-rel
(bf16 rounding; threshold margin ~5x).
"""
import sys
sys.path.insert(0, '/opt/trn_rl_repo')
import numpy as np
import ml_dtypes

from concourse import bass, bacc, tile, mybir, bass_utils

BF16 = mybir.dt.bfloat16
F32 = mybir.dt.float32
AF = mybir.ActivationFunctionType
ALU = mybir.AluOpType

N_CORES = 8
B = 8
T = 512
H = 1024
I = 128
NK = 8
G = 4 * H

_compiled = {}


def _allgather_cols(nc, dpool, sbuf_tile, shard_d, full_cols, n_cores, tag):
    sc = full_cols // n_cores
    bounce_in = dpool.tile([128, sc], BF16, name=f"bi_{tag}")
    bounce_out = dpool.tile([128 * n_cores, sc], BF16, name=f"bo_{tag}")
    nc.gpsimd.dma_start(out=bounce_in[:], in_=shard_d[:])
    nc.gpsimd.collective_compute(
        "AllGather", ALU.bypass,
        replica_groups=[list(range(n_cores))],
        ins=[bounce_in.opt()], outs=[bounce_out.opt()],
    )
    src = bounce_out[:].rearrange("(c p) j -> p c j", c=n_cores, p=128)
    dst = sbuf_tile[:].rearrange("p (c j) -> p c j", c=n_cores, j=sc)
    nc.gpsimd.dma_start(out=dst, in_=src)


def _dedup_ldweights(nc):
    removed = 0
    for bb in nc.main_func.blocks:
        newinsts = []
        last_sig = None
        for ins in bb.instructions:
            tn = type(ins).__name__
            if tn == "InstLdweights":
                sig = repr(ins.ins[0])
                si = ins.sync_info
                clean = True
                if si is not None:
                    ow = getattr(si, "on_wait", None)
                    ou = getattr(si, "on_update", None)
                    if (ow and len(ow)) or (ou and len(ou)):
                        clean = False
                if sig == last_sig and clean:
                    removed += 1
                    continue
                last_sig = sig
            elif tn in ("InstMatmult", "InstMatmultMx"):
                pass
            elif getattr(ins, "engine", None) == mybir.EngineType.PE:
                last_sig = None
            newinsts.append(ins)
        bb.instructions[:] = newinsts
    return removed


def _build_kernel():
    S = (T + 1) * B
    NC = N_CORES
    nc = bacc.Bacc("TRN2", target_bir_lowering=False, debug=False,
                   enable_asserts=True, num_devices=NC)

    xT_d = nc.dram_tensor("xT", [128, T * B], BF16, kind="ExternalInput").ap()
    whh_d = nc.dram_tensor("whh_sh", [128, NK * G // NC], BF16, kind="ExternalInput").ap()
    wih_d = nc.dram_tensor("wih_sh", [128, G // NC], BF16, kind="ExternalInput").ap()
    fcw_d = nc.dram_tensor("fcw_sh", [128, NK * 128 // NC], BF16, kind="ExternalInput").ap()
    bias_d = nc.dram_tensor("bias_rows", [128, G], BF16, kind="ExternalInput").ap()
    fcb_d = nc.dram_tensor("fc_b", [128, 1], F32, kind="ExternalInput").ap()
    out_d = nc.dram_tensor("out_itb", [128, T * B], BF16, kind="ExternalOutput").ap()

    with tile.TileContext(nc) as tc:
        with tc.tile_pool(name="const", bufs=1) as cpool, \
             tc.tile_pool(name="dram", bufs=1, space="DRAM") as dpool:
            xT = cpool.tile([128, T * B], BF16)
            whh = cpool.tile([128, NK * G], BF16)
            wih = cpool.tile([128, G], BF16)
            brows = cpool.tile([128, G], BF16)
            fcw = cpool.tile([128, NK * 128], BF16)
            fcb = cpool.tile([128, 1], F32)
            hh = cpool.tile([128, NK * S], BF16)
            out_sb = cpool.tile([128, T * B], BF16)
            c_sb = cpool.tile([B, H], F32)
            hbf = cpool.tile([16, H], BF16)
            hT = cpool.tile([128, 16 * NK], BF16)

            xp_d = dpool.tile([T * B, G], BF16)

            nc.sync.dma_start(out=xT[:], in_=xT_d[:])
            nc.sync.dma_start(out=brows[:], in_=bias_d[:])
            nc.sync.dma_start(out=fcb[:], in_=fcb_d[:])
            _allgather_cols(nc, dpool, whh, whh_d, NK * G, NC, "whh")
            _allgather_cols(nc, dpool, wih, wih_d, G, NC, "wih")
            _allgather_cols(nc, dpool, fcw, fcw_d, NK * 128, NC, "fcw")

            # xp = x @ W_ih^T + bias -> DRAM (bf16)
            with tc.tile_pool(name="xpps", bufs=1, space="PSUM") as xppool, \
                 tc.tile_pool(name="xpsb", bufs=2) as xsbp:
                for cc in range(T * B // 128):
                    ps = xppool.tile([128, G], F32, tag="xp", name=f"xp{cc}")
                    for n in range(G // 512):
                        nc.tensor.matmul(
                            ps[:, 512 * n:512 * (n + 1)],
                            xT[:, 128 * cc:128 * (cc + 1)],
                            wih[:, 512 * n:512 * (n + 1)],
                            start=True, stop=True)
                    nc.vector.tensor_tensor(out=ps[:], in0=ps[:], in1=brows[:], op=ALU.add)
                    st = xsbp.tile([128, G], BF16, tag="st", name=f"st{cc}")
                    nc.scalar.activation(st[:], ps[:], AF.Copy)
                    nc.sync.dma_start(out=xp_d[128 * cc:128 * (cc + 1), :], in_=st[:])

            nc.vector.memset(c_sb[:], 0.0)
            nc.vector.memset(hbf[:], 0.0)
            hh4 = hh[:].rearrange("p (k s) -> p k s", k=NK, s=S)
            nc.vector.memset(hh4[:, :, 0:B], 0.0)

            # recurrence (gate order i, f, o, g)
            with tc.tile_pool(name="gps", bufs=1, space="PSUM") as gpool, \
                 tc.tile_pool(name="xstp", bufs=2) as xstpool, \
                 tc.tile_pool(name="wk", bufs=1) as wpool:
                for t in range(T):
                    gp = gpool.tile([128, G], F32, tag="g", name=f"g{t}")
                    for k in range(NK):
                        lhsT = hh[:, k * S + t * B:k * S + (t + 1) * B]
                        for n in range(G // 512):
                            nc.tensor.matmul(
                                gp[:B, 512 * n:512 * (n + 1)],
                                lhsT,
                                whh[:, k * G + 512 * n:k * G + 512 * (n + 1)],
                                start=(k == 0), stop=(k == NK - 1))
                    xst = xstpool.tile([B, G], BF16, tag="xst", name=f"xst{t}")
                    nc.sync.dma_start(out=xst[:], in_=xp_d[B * t:B * (t + 1), :])
                    nc.vector.tensor_tensor(out=gp[:B, :], in0=gp[:B, :],
                                            in1=xst[:], op=ALU.add)
                    nc.scalar.activation(gp[:B, 0:3 * H], gp[:B, 0:3 * H], AF.Sigmoid)
                    gsb = wpool.tile([B, H], F32, tag="gsb", name=f"gsb{t}")
                    nc.scalar.activation(gsb[:], gp[:B, 3 * H:4 * H], AF.Tanh)
                    tmp = wpool.tile([B, H], F32, tag="tmp", name=f"tmp{t}")
                    nc.vector.tensor_tensor(out=tmp[:], in0=gp[:B, 0:H],
                                            in1=gsb[:], op=ALU.mult)
                    nc.vector.tensor_tensor(out=c_sb[:], in0=gp[:B, H:2 * H],
                                            in1=c_sb[:], op=ALU.mult)
                    nc.vector.tensor_tensor(out=c_sb[:], in0=c_sb[:], in1=tmp[:],
                                            op=ALU.add)
                    th = wpool.tile([B, H], F32, tag="th", name=f"th{t}")
                    nc.scalar.activation(th[:], c_sb[:], AF.Tanh)
                    nc.vector.tensor_tensor(out=hbf[0:B, :], in0=gp[:B, 2 * H:3 * H],
                                            in1=th[:], op=ALU.mult)
                    for k in range(NK):
                        nc.sync.dma_start_transpose(
                            hT[:, 16 * k:16 * (k + 1)],
                            hbf[:, 128 * k:128 * (k + 1)])
                    hTv = hT[:].rearrange("p (k c) -> p k c", k=NK, c=16)
                    nc.vector.tensor_copy(
                        out=hh4[:, :, (t + 1) * B:(t + 2) * B],
                        in_=hTv[:, :, 0:B])

            # fc over h history: k-outer, 8 live psum banks
            csz = 512
            NCH = T * B // csz
            with tc.tile_pool(name="fcps", bufs=1, space="PSUM") as fpool:
                for blk in range(0, NCH, 8):
                    nb = min(8, NCH - blk)
                    fps = []
                    for j in range(nb):
                        f_t = fpool.tile([128, csz], F32, tag=f"fc{j}",
                                         name=f"fc{blk}_{j}")
                        fps.append(f_t)
                    for k in range(NK):
                        for j in range(nb):
                            n = blk + j
                            nc.tensor.matmul(
                                fps[j][:],
                                fcw[:, 128 * k:128 * (k + 1)],
                                hh[:, k * S + B + csz * n:k * S + B + csz * (n + 1)],
                                start=(k == 0), stop=(k == NK - 1))
                    for j in range(nb):
                        n = blk + j
                        nc.scalar.activation(out_sb[:, csz * n:csz * (n + 1)],
                                             fps[j][:], AF.Identity, bias=fcb[:])

            nc.sync.dma_start(out=out_d[:], in_=out_sb[:])

    nc.compile()
    _dedup_ldweights(nc)
    return nc


def _gate_perm():
    idx = np.arange(G).reshape(4, H)
    return np.concatenate([idx[0], idx[1], idx[3], idx[2]])  # i, f, o, g


def _prep_shared(W_ih, W_hh, b_ih, b_hh, fc_w, fc_b):
    bf = ml_dtypes.bfloat16
    perm = _gate_perm()
    whh_sb = np.ascontiguousarray(
        W_hh[perm].T.reshape(NK, 128, G).transpose(1, 0, 2).reshape(128, NK * G)).astype(bf)
    wih_sb = np.ascontiguousarray(W_ih[perm].T).astype(bf)
    bias = (b_ih + b_hh).astype(np.float32)[perm]
    bias_rows = np.ascontiguousarray(np.broadcast_to(bias, (128, G))).astype(bf)
    fcw_sb = np.ascontiguousarray(
        fc_w.T.reshape(NK, 128, 128).transpose(1, 0, 2).reshape(128, NK * 128)).astype(bf)
    fcb = np.ascontiguousarray(fc_b.reshape(128, 1), dtype=np.float32)
    return whh_sb, wih_sb, bias_rows, fcw_sb, fcb


def _prep_core(x_core, shared, core_id):
    whh_sb, wih_sb, bias_rows, fcw_sb, fcb = shared
    bf = ml_dtypes.bfloat16
    xT = np.ascontiguousarray(x_core.transpose(2, 1, 0).reshape(I, T * B)).astype(bf)

    def shard(a):
        sc = a.shape[1] // N_CORES
        return np.ascontiguousarray(a[:, core_id * sc:(core_id + 1) * sc])

    return {
        "xT": xT,
        "whh_sh": shard(whh_sb),
        "wih_sh": shard(wih_sb),
        "fcw_sh": shard(fcw_sb),
        "bias_rows": bias_rows,
        "fc_b": fcb,
    }


def kernel(x, W_ih, W_hh, b_ih, b_hh, fc_w, fc_b):
    x = np.asarray(x, dtype=np.float32)
    args = [np.asarray(a, dtype=np.float32)
            for a in (W_ih, W_hh, b_ih, b_hh, fc_w, fc_b)]

    if "nc" not in _compiled:
        _compiled["nc"] = _build_kernel()
    nc = _compiled["nc"]

    shared = _prep_shared(*args)
    in_maps = [_prep_core(x[c * B:(c + 1) * B], shared, c) for c in range(N_CORES)]
    res = bass_utils.run_bass_kernel_spmd(nc, in_maps, core_ids=list(range(N_CORES)))

    out = np.empty((N_CORES * B, T, I), dtype=np.float32)
    for c in range(N_CORES):
        co = np.asarray(res.results[c]["out_itb"], dtype=np.float32)
        out[c * B:(c + 1) * B] = co.reshape(I, T, B).transpose(2, 1, 0)
    return out

